# revision 11
# baseline (speedup 1.0000x reference)
"""PiCANet-G attention module as a Trainium2 Bass/Tile kernel.

Pure data-parallel over batch: 64 samples -> 8 cores x 8 samples.

Per core, three phases (all SBUF-resident):
  P1: vertical bi-LSTM over W (batch = 8*28 (b, h) rows, 28 steps, 2 dirs)
  P2: horizontal bi-LSTM over H (batch = 8*28 (b, w) rows)
  P3: fc -> softmax(100) -> per-sample einsum with the dilated 10x10 patch

Engine split per (step, dir), designed from the TimelineSim cost model:
  PE  : fp8 DoubleRow matmuls, gates in one PSUM tile [128,4,512]
        (rows i,f,o,g), 2 dirs ping-pong the 8 PSUM banks.
  Act : ONE sigmoid instruction over the i,f,o rows -> fp16 sigm tile.
        (tanh is NOT on Act - it moves to DVE custom ops.)
  DVE : custom deg-5 clamped-tanh op (TANH5C, 8 ALU stages) for tanh(g)
        and tanh(c); one fused 896-elem mul computes t1=sig_i*th_g and
        cf=sig_f*c together; one 448-elem add updates c.
  Pool: h = sig_o * tanh(c) written as fp8 straight into the h-slab
        (strided for P1's (b,h,w) slab - no mirror copies needed).

The Hv slab is (kt, b, h, w) with w innermost so that P1's per-step h
write is a stride-28 slice and P2's per-step ih rhs is a [p,2,8,28]
3-free-dim AP (both verified supported).  Hh stays (h,(b,w)) so P2's
writes and P3's fc reads are contiguous.
"""

import numpy as np
import ml_dtypes
from contextlib import ExitStack

import concourse.bacc as bacc
import concourse.mybir as mybir
import concourse.tile as tile
from concourse.masks import make_identity
from concourse.bass_utils import run_bass_kernel_spmd

# ---- custom DVE op: clamped deg-5 odd-poly tanh ---------------------------
from concourse import dve_ops
from concourse.dve_ops import DveOp, _SUB_OPCODE_FOR_NAME
from concourse.dve_spec import (Spec, Src0, C0, C1, C2, Zero, One, minn, maxx,
                                sq, lower)
from concourse.dve_uop import DveOpSpec

# deg-5 odd clamped-tanh fits. Bias-corrected (zero density-weighted mean
# error over the empirical pre-activation / cell distributions) — the
# systematic part of the poly error compounds through the recurrence, so a
# zero-mean fit beats pure minimax end-to-end. Poly stays >= 1 after
# crossing, so the clamp holds for all |x|.
TPG = (0.92440501, -0.17230076, 0.0147892)    # tanh(g) pre-activation path
TPC = (0.97490551, -0.21231092, 0.02164544)   # tanh(c) cell path


def _register_tanh5c():
    name = "TANH5C_ANT"
    if name in _SUB_OPCODE_FOR_NAME:
        return next(op for op in dve_ops.OPS if op.name == name)
    t = sq(Src0)
    h = (C2 * t + C1) * t + C0
    body = maxx(minn(h * Src0, One), Zero - One)
    spec = Spec(body=body,
                reference=lambda in0, in1, s0, s1, imm2: np.clip(
                    in0 * (s0 + s1 * in0**2 + imm2 * in0**4), -1, 1
                ).astype(np.float32))
    shas = {}
    for ver in ("v3", "v4"):
        uops = lower(spec, ver=ver)
        s = DveOpSpec(name=name, opcode=0, uops=uops,
                      rd1_en=dve_ops.has_src1(spec))
        shas[ver] = s.sha(ver)
    op = DveOp(name, spec, False, shas)
    row = max(_SUB_OPCODE_FOR_NAME.values()) + 1
    assert row < 0x20
    dve_ops.OPS.append(op)
    dve_ops.CUSTOM_DVE_SPECS[name] = spec
    _SUB_OPCODE_FOR_NAME[name] = row
    return op


TANH5C = _register_tanh5c()

# problem shapes (hardcoded per contract)
B, C, H, W = 64, 512, 28, 28
HID = 256
N_CORES = 8
BL = B // N_CORES        # samples per core
NB = BL * H              # 224 rows per LSTM step
T = 28                   # steps per LSTM
PLOC = BL * H * W        # 6272 positions per core

BF16 = mybir.dt.bfloat16
F32 = mybir.dt.float32
F16 = mybir.dt.float16
F8 = mybir.dt.float8e4
AF = mybir.ActivationFunctionType
DR = mybir.MatmulPerfMode.DoubleRow

# torch gate order [i f g o] -> device order [i f o g] (sigmoids first)
_PERM = np.concatenate([np.arange(0, 512), np.arange(768, 1024), np.arange(512, 768)])

_LSTMS = ["vf", "vb", "hf", "hb"]


def _emit_matmuls(nc, pd, wih_sb, whh_sb, src_rhs, hprev_rhs, t):
    """PE work for one (step, dir): per (gate, half) m-tile, an accumulation
    group of 2 fp8 DoubleRow ih matmuls (+1 hh when t>0)."""
    for gate in (0, 1, 2, 3):
        for hhalf in range(2):
            m = gate * 2 + hhalf
            out_ap = pd[:, gate, hhalf * 256: hhalf * 256 + 224]
            for q in range(2):
                nc.tensor.matmul(
                    out_ap,
                    lhsT=wih_sb[:, 2 * q:2 * q + 2, m * 128:(m + 1) * 128],
                    rhs=src_rhs(q),
                    start=(q == 0), stop=(t == 0 and q == 1),
                    perf_mode=DR)
            if t > 0:
                nc.tensor.matmul(
                    out_ap,
                    lhsT=whh_sb[:, 0:2, m * 128:(m + 1) * 128],
                    rhs=hprev_rhs,
                    start=False, stop=True, perf_mode=DR)


def _emit_cells(nc, scr, pds, THs, THns, t, names):
    """Act sigmoid + DVE tanh/cell for BOTH dirs of one step, emitted
    stage-interleaved so each engine's in-order stream alternates dirs
    (the two recurrence chains hide each other's latency).

    THs[d] = [128, 2, 2, 224] f16: [:,0] th_g(t), [:,1] c(t-1).
    THns[d] = next step's tile: the add writes c(t) into THns[d][:,1].
    Returns (sigms, th_cs)."""
    gvs = [pd.rearrange("p g (h x) -> p g h x", h=2) for pd in pds]
    sigms, th_cs = [], []
    for d in range(2):
        sigm = scr.tile([128, 3, 2, 224], F16, tag=f"sg{d}", bufs=3,
                        name=f"sg_{names[d]}")
        nc.scalar.activation(sigm, gvs[d][:, 0:3, :, 0:224], AF.Sigmoid)
        sigms.append(sigm)
        nc.vector._custom_dve(TANH5C, out=THs[d][:, 0],
                              in0=gvs[d][:, 3, :, 0:224],
                              s0=TPG[0], s1=TPG[1], imm2=TPG[2])
        if t == 0:
            nc.vector.tensor_mul(THns[d][:, 1], sigms[d][:, 0], THs[d][:, 0])
        else:
            ot = scr.tile([128, 2, 2, 224], F16, tag=f"ot{d}", bufs=3,
                          name=f"ot_{names[d]}")
            nc.vector.tensor_mul(ot, sigms[d][:, 0:2], THs[d])
            nc.vector.tensor_add(THns[d][:, 1], ot[:, 0], ot[:, 1])
        th_c = scr.tile([128, 2, 224], F16, tag=f"tc{d}", bufs=3,
                        name=f"tc_{names[d]}")
        nc.vector._custom_dve(TANH5C, out=th_c, in0=THns[d][:, 1],
                              s0=TPC[0], s1=TPC[1], imm2=TPC[2])
        th_cs.append(th_c)
    return sigms, th_cs


def _build(reps=1, debug=False):
    nc = bacc.Bacc(None, target_bir_lowering=False)

    xT_d = nc.dram_tensor("xT", [C, PLOC], F8, kind="ExternalInput")
    w_d = {}
    for L in _LSTMS:
        w_d[L + "_wih"] = nc.dram_tensor(L + "_wih", [512, 1024], F8, kind="ExternalInput")
        w_d[L + "_whh"] = nc.dram_tensor(L + "_whh", [256, 1024], F8, kind="ExternalInput")
    fcw_d = nc.dram_tensor("fcw", [512, 100], F8, kind="ExternalInput")
    patchT_d = nc.dram_tensor("patchT", [BL, 100, 512], BF16, kind="ExternalInput")
    out_d = nc.dram_tensor("out", [BL, C, H * W], BF16, kind="ExternalOutput")
    if debug:
        dbg_hv = nc.dram_tensor("dbg_hv", [128, 4, PLOC], F8, kind="ExternalOutput")
        dbg_hh = nc.dram_tensor("dbg_hh", [128, 4, PLOC], F8, kind="ExternalOutput")

    with tile.TileContext(nc) as tc, ExitStack() as ctx:
        wpool = ctx.enter_context(tc.tile_pool(name="wpool", bufs=1))
        bigA = ctx.enter_context(tc.tile_pool(name="bigA", bufs=1))
        bigB = ctx.enter_context(tc.tile_pool(name="bigB", bufs=1))
        state = ctx.enter_context(tc.tile_pool(name="state", bufs=1))
        scr = ctx.enter_context(tc.tile_pool(name="scr", bufs=3))

        # --- load weights; both stage-1 dirs first (step 0 needs them) ---
        wih_sb, whh_sb = {}, {}
        for L in _LSTMS:
            wih_sb[L] = wpool.tile([128, 4, 1024], F8, name=f"wih_{L}")
            whh_sb[L] = wpool.tile([128, 2, 1024], F8, name=f"whh_{L}")
        vf_src = w_d["vf_wih"].rearrange("(kt p) m -> p kt m", kt=4)
        vb_src = w_d["vb_wih"].rearrange("(kt p) m -> p kt m", kt=4)
        nc.sync.dma_start(out=wih_sb["vf"][:, 0:2], in_=vf_src[:, 0:2])
        nc.sync.dma_start(out=wih_sb["vb"][:, 0:2], in_=vb_src[:, 0:2])
        nc.sync.dma_start(out=wih_sb["vf"][:, 2:4], in_=vf_src[:, 2:4])
        nc.gpsimd.dma_start(out=wih_sb["vb"][:, 2:4], in_=vb_src[:, 2:4])
        for L in ["vf", "vb"]:
            nc.gpsimd.dma_start(out=whh_sb[L],
                                in_=w_d[L + "_whh"].rearrange("(kt p) m -> p kt m", kt=2))
        for L in ["hf", "hb"]:
            nc.sync.dma_start(out=wih_sb[L],
                              in_=w_d[L + "_wih"].rearrange("(kt p) m -> p kt m", kt=4))
            nc.sync.dma_start(out=whh_sb[L],
                              in_=w_d[L + "_whh"].rearrange("(kt p) m -> p kt m", kt=2))
        fcw_sb = wpool.tile([128, 4, 100], F8, name="fcw_sb")
        nc.sync.dma_start(out=fcw_sb, in_=fcw_d.rearrange("(kt p) n -> p kt n", kt=4))
        patchT_sb = wpool.tile([100, BL, 512], BF16, name="patchT_sb")
        nc.sync.dma_start(out=patchT_sb, in_=patchT_d.rearrange("b k c -> k b c"))
        ident = wpool.tile([112, 112], F32, name="ident")
        make_identity(nc, ident)
        # warm the Act LUTs (sigmoid + exp) during the DMA ramp
        warm = wpool.tile([128, 2], F32, name="warm")
        nc.vector.memset(warm, 0.0)
        nc.scalar.activation(warm[:, 0:1], warm[:, 0:1], AF.Sigmoid)
        nc.scalar.activation(warm[:, 1:2], warm[:, 1:2], AF.Exp)

        for rep in range(reps):
            sfx = f"r{rep}"
            # --- P1: vertical bi-LSTM ---
            # input cols (w, b, h); output slab Hv in (kt, b, h, w)
            xT = bigA.tile([128, 4, PLOC], F8, tag="bigA", name=f"xT_{sfx}")
            xsrc = xT_d.rearrange("(kt p) f -> p kt f", kt=4)
            for eng, blocks in [(nc.scalar, [(0, 1), (27, 28), (1, 5), (23, 27),
                                             (5, 11)]),
                                (nc.gpsimd, [(17, 23), (11, 17)])]:
                for lo, hi in blocks:
                    eng.dma_start(out=xT[:, :, lo * 224:hi * 224],
                                  in_=xsrc[:, :, lo * 224:hi * 224])
            Hv = bigB.tile([128, 4, PLOC], F8, tag="bigB", name=f"Hv_{sfx}")
            Hv5 = Hv.rearrange("p kt (b h w) -> p kt b h w", b=BL, h=H)

            with tc.tile_pool(name=f"g1{sfx}", bufs=1, space="PSUM") as gpool:
                THn = [None, None]
                for t in range(T):
                    poss = [t, T - 1 - t]
                    pds = [gpool.tile([128, 4, 512], F32, tag=f"pd{d}",
                                      name=f"pd1_{d}_{t}_{sfx}")
                           for d in range(2)]
                    for d, L in enumerate(["vf", "vb"]):
                        ppos = t - 1 if d == 0 else T - t
                        hprev = (None if t == 0 else
                                 Hv5[:, 2 * d:2 * d + 2, :, :, ppos])
                        _emit_matmuls(nc, pds[d], wih_sb[L], whh_sb[L],
                                      lambda q, _p=poss[d]: xT[:, 2 * q:2 * q + 2,
                                                               _p * 224:(_p + 1) * 224],
                                      hprev, t)
                    names = [f"1{d}_{t}_{sfx}" for d in range(2)]
                    THs = []
                    for d in range(2):
                        THs.append(THn[d] if t > 0 else scr.tile(
                            [128, 2, 2, 224], F16, tag=f"TH{d}", bufs=2,
                            name=f"TH_{names[d]}"))
                        THn[d] = scr.tile([128, 2, 2, 224], F16, tag=f"TH{d}",
                                          bufs=2, name=f"THn_{names[d]}")
                    sigms, th_cs = _emit_cells(nc, scr, pds, THs, THn, t, names)
                    for d in range(2):
                        dst = Hv5[:, 2 * d:2 * d + 2, :, :, poss[d]]
                        nc.gpsimd.tensor_mul(
                            dst,
                            sigms[d][:, 2].rearrange("p h (b x) -> p h b x", b=BL),
                            th_cs[d].rearrange("p h (b x) -> p h b x", b=BL))

            # --- P2: horizontal bi-LSTM ---
            # P2 in: Hv (b,h,w); out slab Hh in (h, (b,w)) h-major
            Hh = bigA.tile([128, 4, PLOC], F8, tag="bigA", name=f"Hh_{sfx}")
            KT = bigB.tile([100, PLOC], BF16, tag="bigB2", name=f"KT_{sfx}")
            with tc.tile_pool(name=f"g2{sfx}", bufs=1, space="PSUM") as gpool:
                THn = [None, None]
                for t in range(T):
                    poss = [t, T - 1 - t]
                    pds = [gpool.tile([128, 4, 512], F32, tag=f"pd{d}",
                                      name=f"pd2_{d}_{t}_{sfx}")
                           for d in range(2)]
                    for d, L in enumerate(["hf", "hb"]):
                        ppos = t - 1 if d == 0 else T - t
                        hprev = (None if t == 0 else
                                 Hh[:, 2 * d:2 * d + 2,
                                    ppos * 224:(ppos + 1) * 224])
                        _emit_matmuls(nc, pds[d], wih_sb[L], whh_sb[L],
                                      lambda q, _p=poss[d]: Hv.rearrange(
                                          "p kt (b h w) -> p kt b h w",
                                          b=BL, h=H)[:, 2 * q:2 * q + 2, :, _p, :],
                                      hprev, t)
                    names = [f"2{d}_{t}_{sfx}" for d in range(2)]
                    THs = []
                    for d in range(2):
                        THs.append(THn[d] if t > 0 else scr.tile(
                            [128, 2, 2, 224], F16, tag=f"TH{d}", bufs=2,
                            name=f"TH_{names[d]}"))
                        THn[d] = scr.tile([128, 2, 2, 224], F16, tag=f"TH{d}",
                                          bufs=2, name=f"THn_{names[d]}")
                    sigms, th_cs = _emit_cells(nc, scr, pds, THs, THn, t, names)
                    for d in range(2):
                        hslice = Hh[:, 2 * d:2 * d + 2,
                                    poss[d] * 224:(poss[d] + 1) * 224]
                        nc.gpsimd.tensor_mul(hslice, sigms[d][:, 2], th_cs[d])

            # --- P3: fc + softmax + transpose + einsum (own PSUM scope) ---
            with tc.tile_pool(name=f"p3{sfx}", bufs=2, space="PSUM") as pps:
                hq_order = sorted(range(0, H, 4), key=lambda h: max(27 - h, h + 3))
                for half in range(2):
                    for hq in hq_order:
                        unit = {"half": half, "hq": hq}
                        _emit_p3_unit(nc, scr, pps, Hh, KT, fcw_sb, ident,
                                      unit, sfx)
                    # einsum for this half's samples overlaps the other
                    # half's softmax pipeline (single shared PSUM pool)
                    for b_i in range(half * 4, (half + 1) * 4):
                        for ch in range(2):
                            ob = scr.tile([128, 2, 2, 392], BF16, tag="ob", bufs=8,
                                          name=f"ob_{b_i}_{ch}_{sfx}")
                            for c2 in range(2):
                                ct = ch * 2 + c2
                                lhsT = patchT_sb[:, b_i, ct * 128:(ct + 1) * 128]
                                for j2 in range(2):
                                    Op = pps.tile([128, 512], F32, tag="O", bufs=4,
                                                  name=f"O_{b_i}_{ct}_{j2}_{sfx}")
                                    nc.tensor.matmul(
                                        Op[:, 0:392], lhsT=lhsT,
                                        rhs=KT[:, b_i * 784 + j2 * 392:
                                               b_i * 784 + (j2 + 1) * 392],
                                        start=True, stop=True)
                                    if (ct * 2 + j2) % 2 == 0:
                                        nc.vector.tensor_copy(ob[:, c2, j2], Op[:, 0:392])
                                    else:
                                        nc.scalar.copy(ob[:, c2, j2], Op[:, 0:392])
                            if b_i >= 5:
                                eng = [nc.sync, nc.gpsimd, nc.scalar][(b_i * 2 + ch) % 3]
                            else:
                                eng = [nc.sync, nc.gpsimd][(b_i * 2 + ch) % 2]
                            eng.dma_start(
                                out=out_d[b_i, ch * 256:(ch + 1) * 256, :].rearrange(
                                    "(ct p) f -> p ct f", ct=2),
                                in_=ob.rearrange("p ct two f -> p ct (two f)"))
            if debug and rep == reps - 1:
                nc.sync.dma_start(out=dbg_hv[:, :, :], in_=Hv)
                nc.sync.dma_start(out=dbg_hh[:, :, :], in_=Hh)

    nc.compile()
    return nc


def _emit_p3_unit(nc, scr, pps, Hh, KT, fcw_sb, ident, unit, sfx):
    """fc -> exp -> rowsum -> reciprocal -> normalize -> transpose -> KT.
    Standalone phase: Act has slack here, so normalize and the KT copy are
    spread over DVE / Act / Pool as in the baseline."""
    half, hq = unit["half"], unit["hq"]
    name = f"{hq}_{half}_{sfx}"
    Lp = pps.tile([112, 4, 100], F32, tag="L", bufs=2, name=f"L_{name}")
    for r in range(4):
        off = (hq + r) * 224 + half * 112
        for q in range(2):
            nc.tensor.matmul(Lp[:, r],
                             lhsT=Hh[:, 2 * q:2 * q + 2, off:off + 112],
                             rhs=fcw_sb[:, 2 * q:2 * q + 2, :],
                             start=(q == 0), stop=(q == 1),
                             perf_mode=DR)
    E = scr.tile([112, 4, 100], F32, tag="E", bufs=4, name=f"E_{name}")
    nc.scalar.activation(E, Lp, AF.Exp)
    Zs = scr.tile([112, 4], F32, tag="Z", bufs=4, name=f"Z_{name}")
    nc.vector.reduce_sum(Zs, E, axis=mybir.AxisListType.X)
    rz = scr.tile([112, 4], F32, tag="rz", bufs=4, name=f"rz_{name}")
    nc.vector.reciprocal(rz, Zs)
    Ka = scr.tile([112, 4, 100], F32, tag="Ka", bufs=4, name=f"Ka_{name}")
    nc.vector.tensor_scalar_mul(Ka[:, 0], E[:, 0], rz[:, 0:1])
    nc.scalar.activation(Ka[:, 1], E[:, 1], AF.Copy, scale=rz[:, 1:2])
    for r in (2, 3):
        nc.gpsimd.tensor_scalar_mul(Ka[:, r], E[:, r], rz[:, r:r + 1])
    KTp = pps.tile([100, 4, 112], F32, tag="KTp", bufs=2, name=f"KTp_{name}")
    for r in range(4):
        nc.tensor.transpose(KTp[:, r], Ka[:, r], ident)
    dst = KT.rearrange("k (b q hw) -> k b q hw", b=BL,
                       q=7)[:, half * 4:(half + 1) * 4, hq // 4, :]
    dstv = dst.rearrange("k b (r w) -> k b r w", r=4)
    src = KTp.rearrange("k r (b w) -> k b r w", b=4)
    if (hq // 4 + half) % 2 == 0:
        nc.vector.tensor_copy(dstv, src)
    else:
        nc.scalar.copy(dstv, src)


_NC_CACHE = {}


def _get_nc(reps=1, debug=False):
    key = (reps, debug)
    if key not in _NC_CACHE:
        _NC_CACHE[key] = _build(reps=reps, debug=debug)
    return _NC_CACHE[key]


def _prep_core_inputs(x, weights_np):
    """Host-side marshalling for one core. x: [BL, C, H, W] f32."""
    f8 = ml_dtypes.float8_e4m3
    bf = ml_dtypes.bfloat16
    m = {}
    m["xT"] = np.ascontiguousarray(
        x.transpose(1, 3, 0, 2).reshape(C, PLOC)).astype(f8)
    m["patchT"] = np.ascontiguousarray(
        x[:, :, ::3, ::3].reshape(BL, C, 100).transpose(0, 2, 1)).astype(bf)
    m.update(weights_np)
    return m


def _prep_weights(inputs):
    f8 = ml_dtypes.float8_e4m3
    w = {}
    for L in _LSTMS:
        wih = np.asarray(inputs[L + "_Wih"], np.float32)
        whh = np.asarray(inputs[L + "_Whh"], np.float32)
        w[L + "_wih"] = np.ascontiguousarray(wih[_PERM].T).astype(f8)
        w[L + "_whh"] = np.ascontiguousarray(whh[_PERM].T).astype(f8)
    w["fcw"] = np.asarray(inputs["fc_W"], np.float32).astype(f8)
    return w


def run_cores(inputs, reps=1, debug=False):
    x = np.asarray(inputs["x"], np.float32)
    wnp = _prep_weights(inputs)
    nc = _get_nc(reps=reps, debug=debug)
    in_maps = [
        _prep_core_inputs(x[ci * BL:(ci + 1) * BL], wnp) for ci in range(N_CORES)
    ]
    res = run_bass_kernel_spmd(nc, in_maps, list(range(N_CORES)))
    return res


def kernel(**inputs) -> np.ndarray:
    res = run_cores(inputs)
    out = np.concatenate(
        [res.results[ci]["out"].reshape(BL, C, H, W) for ci in range(N_CORES)],
        axis=0)
    return out.astype(np.float32)
